# revision 9
# baseline (speedup 1.0000x reference)
"""Trainium2 Bass kernel for AttnBlock (GroupNorm + QKV + NxN attention + proj + residual).

Contract: kernel(**inputs) takes the FULL unsharded inputs (as produced by
setup_inputs) and returns the FULL output, running on 8 NeuronCores via
bass_utils.run_bass_kernel_spmd.

Sharding: core i handles (batch b = i//4, query-shard s = i%4). The host
rotates x[b] by -s*1024 along the flattened spatial axis so the (identical)
SPMD program always treats columns 0:1024 as its query rows: attention and
GroupNorm are permutation-invariant over key positions, so only the output
column order matters, and out columns 0:1024 of the rotated problem are
exactly out[b][:, s*1024:(s+1)*1024] of the original.

On-chip layout (per core):
  - channels on partitions in 2 halves of 128; spatial (4096) on the free axis
  - GroupNorm stats via bn_stats/bn_aggr per channel, then one block-diagonal
    (1/8) 128x128 fp32 matmul to average the 8 channels of each group, rstd on
    DVE/ACT, applied with one ACT pass (per-partition scale+bias)
  - q/k/v weights are passed pre-transposed (c,o) so all matmuls are
    transpose-free; v is produced directly in (m, c) layout
  - scores are computed transposed, S^T[m,n] = sum_c k[c,m] q[c,n], with keys
    m on partitions (32 chunks of 128): softmax over m needs no max pass
    (|scores| <~ 10), exp runs on ACT into fp16, PV accumulates
    h[c,n] += vT[m,c]^T exp[m,n] in PSUM across all 32 chunks
  - softmax denominator: DVE accumulates exp chunks, then a ones(128,128)
    matmul computes column sums broadcast to all partitions in one shot;
    h is scaled by the reciprocal after PV (division commutes with PV)
  - bv is folded into the h finalize (softmax rows sum to one); bp and the
    residual add are fused into a single scalar_tensor_tensor
"""

import numpy as np

C = 256
N = 4096  # spatial positions (16*16*16)
NSH = 1024  # query shard per core
NCORES = 8
EPS = 1e-6
SCALE = 1.0 / 16.0  # C ** -0.5

_CACHE = {}


def _build_program():
    import concourse.bass as bass
    import concourse.tile as tile
    from concourse import bacc, mybir

    F32 = mybir.dt.float32
    F16 = mybir.dt.float16
    Alu = mybir.AluOpType
    Act = mybir.ActivationFunctionType

    nc = bacc.Bacc("TRN2", target_bir_lowering=False, debug=False,
                   num_devices=NCORES)

    d_xb = nc.dram_tensor("xb", [2, 128, N], F32, kind="ExternalInput").ap()
    d_wqT = nc.dram_tensor("wqT", [2, 128, C], F16, kind="ExternalInput").ap()
    d_wkT = nc.dram_tensor("wkT", [2, 128, C], F16, kind="ExternalInput").ap()
    d_wvT = nc.dram_tensor("wvT", [2, 128, C], F16, kind="ExternalInput").ap()
    d_wpT = nc.dram_tensor("wpT", [2, 128, C], F16, kind="ExternalInput").ap()
    d_gamma = nc.dram_tensor("gamma", [2, 128, 1], F32, kind="ExternalInput").ap()
    d_beta = nc.dram_tensor("beta", [2, 128, 1], F32, kind="ExternalInput").ap()
    d_bq = nc.dram_tensor("bq", [2, 128, 1], F32, kind="ExternalInput").ap()
    d_bk = nc.dram_tensor("bk", [2, 128, 1], F32, kind="ExternalInput").ap()
    d_bv = nc.dram_tensor("bv", [2, 128, 1], F32, kind="ExternalInput").ap()
    d_bp = nc.dram_tensor("bp", [2, 128, 1], F32, kind="ExternalInput").ap()
    d_gmat = nc.dram_tensor("gmat", [128, 128], F32, kind="ExternalInput").ap()
    d_ones = nc.dram_tensor("ones", [128, 128], F16, kind="ExternalInput").ap()
    d_out = nc.dram_tensor("out", [2, 128, NSH], F32, kind="ExternalOutput").ap()

    MCH = N // 128  # 32 key chunks

    with tile.TileContext(nc) as tc:
        with (
            tc.tile_pool(name="persist", bufs=1) as P,
            tc.tile_pool(name="work", bufs=2) as W,
            tc.tile_pool(name="psum", bufs=1, space="PSUM") as PS,
        ):
            # ---- constants / weights ----
            gmat = P.tile([128, 128], F32, tag="gmat")
            nc.sync.dma_start(out=gmat, in_=d_gmat)
            ones = P.tile([128, 128], F16, tag="ones")
            nc.sync.dma_start(out=ones, in_=d_ones)

            def load_cols(dram, tag, dt=F32):
                ts = []
                for h in range(2):
                    t = P.tile([128, 1], dt, tag=f"{tag}{h}")
                    nc.sync.dma_start(out=t, in_=dram[h])
                    ts.append(t)
                return ts

            gamma = load_cols(d_gamma, "gamma")
            beta = load_cols(d_beta, "beta")
            bq = load_cols(d_bq, "bq")
            bk = load_cols(d_bk, "bk")
            bv = load_cols(d_bv, "bv")
            bp = load_cols(d_bp, "bp")

            def load_w(dram, tag):
                ts = []
                for h in range(2):
                    t = P.tile([128, C], F16, tag=f"{tag}{h}")
                    nc.sync.dma_start(out=t, in_=dram[h])
                    ts.append(t)
                return ts

            wqT = load_w(d_wqT, "wqT")
            wkT = load_w(d_wkT, "wkT")
            wvT = load_w(d_wvT, "wvT")
            wpT = load_w(d_wpT, "wpT")

            eps_t = P.tile([128, 1], F32, tag="eps")
            nc.vector.memset(eps_t, EPS)

            # ---- load x (per half, 4 column chunks for DMA/stats overlap) ----
            xb = []
            for h in range(2):
                t = P.tile([128, N], F32, tag=f"xb{h}")
                for j in range(4):
                    nc.sync.dma_start(
                        out=t[:, j * 1024:(j + 1) * 1024],
                        in_=d_xb[h, :, j * 1024:(j + 1) * 1024],
                    )
                xb.append(t)

            # ---- GroupNorm stats ----
            hn = []
            for h in range(2):
                stats = W.tile([128, 8, 6], F32, tag="bnstats", bufs=2)
                for j in range(8):
                    nc.vector.bn_stats(
                        out=stats[:, j, :], in_=xb[h][:, j * 512:(j + 1) * 512]
                    )
                mv = W.tile([128, 2], F32, tag="mv", bufs=2)
                nc.vector.bn_aggr(out=mv, in_=stats)

                # cm = [mean, mean^2 + var] per channel
                cm = W.tile([128, 2], F32, tag="cm", bufs=2)
                nc.vector.tensor_copy(out=cm[:, 0:1], in_=mv[:, 0:1])
                nc.vector.scalar_tensor_tensor(
                    out=cm[:, 1:2], in0=mv[:, 0:1], scalar=mv[:, 0:1],
                    in1=mv[:, 1:2], op0=Alu.mult, op1=Alu.add,
                )
                # per-channel group stats: [mean_g, E_g[x^2]] (gmat has the /8)
                gst = PS.tile([128, 2], F32, tag="st", bufs=2)
                nc.tensor.matmul(gst, gmat, cm)

                gsb = W.tile([128, 2], F32, tag="gsb", bufs=2)
                nc.vector.tensor_copy(out=gsb, in_=gst)
                msq = W.tile([128, 1], F32, tag="msq", bufs=2)
                nc.vector.tensor_mul(out=msq, in0=gsb[:, 0:1], in1=gsb[:, 0:1])
                varg = W.tile([128, 1], F32, tag="varg", bufs=2)
                nc.vector.tensor_sub(out=varg, in0=gsb[:, 1:2], in1=msq)
                sd = W.tile([128, 1], F32, tag="sd", bufs=2)
                nc.scalar.activation(out=sd, in_=varg, func=Act.Sqrt,
                                     bias=eps_t, scale=1.0)
                rstd = W.tile([128, 1], F32, tag="rstd", bufs=2)
                nc.vector.reciprocal(out=rstd, in_=sd)
                s_t = P.tile([128, 1], F32, tag=f"s{h}")
                nc.vector.tensor_mul(out=s_t, in0=rstd, in1=gamma[h])
                ms = W.tile([128, 1], F32, tag="ms", bufs=2)
                nc.vector.tensor_mul(out=ms, in0=gsb[:, 0:1], in1=s_t)
                t_t = P.tile([128, 1], F32, tag=f"t{h}")
                nc.vector.tensor_sub(out=t_t, in0=beta[h], in1=ms)

                # apply: hn = x * s + t  (fp16)
                ht = P.tile([128, N], F16, tag=f"hn{h}")
                for j in range(2):
                    nc.scalar.activation(
                        out=ht[:, j * 2048:(j + 1) * 2048],
                        in_=xb[h][:, j * 2048:(j + 1) * 2048],
                        func=Act.Identity, bias=t_t, scale=s_t,
                    )
                hn.append(ht)

            # ---- q (only shard columns 0:NSH) ----
            q_sb = []
            for oh in range(2):
                qp = PS.tile([128, NSH], F32, tag="st", bufs=2)
                for nh in range(2):
                    for ch in range(2):
                        nc.tensor.matmul(
                            qp[:, nh * 512:(nh + 1) * 512],
                            wqT[ch][:, oh * 128:(oh + 1) * 128],
                            hn[ch][:, nh * 512:(nh + 1) * 512],
                            start=(ch == 0), stop=(ch == 1),
                        )
                qs = P.tile([128, NSH], F16, tag=f"q{oh}")
                nc.scalar.activation(out=qs, in_=qp, func=Act.Identity, bias=bq[oh])
                q_sb.append(qs)

            # ---- k (full 4096) ----
            k_sb = []
            for oh in range(2):
                ks = P.tile([128, N], F16, tag=f"k{oh}")
                for mt in range(4):
                    kp = PS.tile([128, 1024], F32, tag="st", bufs=2)
                    for nh in range(2):
                        for ch in range(2):
                            nc.tensor.matmul(
                                kp[:, nh * 512:(nh + 1) * 512],
                                wkT[ch][:, oh * 128:(oh + 1) * 128],
                                hn[ch][:, mt * 1024 + nh * 512:
                                        mt * 1024 + (nh + 1) * 512],
                                start=(ch == 0), stop=(ch == 1),
                            )
                    nc.scalar.activation(
                        out=ks[:, mt * 1024:(mt + 1) * 1024], in_=kp,
                        func=Act.Identity, bias=bk[oh],
                    )
                k_sb.append(ks)

            # ---- vT: (m, c) layout, computed directly (no bias: folded later)
            vt = P.tile([128, MCH * C], F16, tag="vt")
            for mc in range(MCH):
                vp = PS.tile([128, C], F32, tag="st", bufs=2)
                for ch in range(2):
                    nc.tensor.matmul(
                        vp, hn[ch][:, mc * 128:(mc + 1) * 128], wvT[ch],
                        start=(ch == 0), stop=(ch == 1),
                    )
                nc.vector.tensor_copy(out=vt[:, mc * C:(mc + 1) * C], in_=vp)

            # ---- attention: S^T chunks, exp, denominator acc, PV ----
            dacc = P.tile([128, NSH], F16, tag="dacc")
            h_ps = [PS.tile([128, NSH], F32, tag=f"h{ch}", bufs=1,
                            name=f"h_ps{ch}")
                    for ch in range(2)]
            for mc in range(MCH):
                st = PS.tile([128, NSH], F32, tag="st", bufs=2)
                for nh in range(2):
                    for ch in range(2):
                        nc.tensor.matmul(
                            st[:, nh * 512:(nh + 1) * 512],
                            k_sb[ch][:, mc * 128:(mc + 1) * 128],
                            q_sb[ch][:, nh * 512:(nh + 1) * 512],
                            start=(ch == 0), stop=(ch == 1),
                        )
                ex = W.tile([128, NSH], F16, tag="ex", bufs=3)
                nc.scalar.activation(out=ex, in_=st, func=Act.Exp, scale=SCALE)
                if mc == 0:
                    nc.vector.tensor_copy(out=dacc, in_=ex)
                else:
                    nc.vector.tensor_add(out=dacc, in0=dacc, in1=ex)
                for ch in range(2):
                    for nh in range(2):
                        nc.tensor.matmul(
                            h_ps[ch][:, nh * 512:(nh + 1) * 512],
                            vt[:, mc * C + ch * 128: mc * C + (ch + 1) * 128],
                            ex[:, nh * 512:(nh + 1) * 512],
                            start=(mc == 0), stop=(mc == MCH - 1),
                        )

            # ---- denominator: colsum broadcast to all partitions, reciprocal
            den = PS.tile([128, NSH], F32, tag="st", bufs=2)
            for nh in range(2):
                nc.tensor.matmul(den[:, nh * 512:(nh + 1) * 512], ones,
                                 dacc[:, nh * 512:(nh + 1) * 512])
            recip = P.tile([128, NSH], F32, tag="recip")
            nc.vector.reciprocal(out=recip, in_=den)

            # ---- h finalize: h * recip + bv  (fp16 for the wp matmul) ----
            hf = []
            for ch in range(2):
                t = P.tile([128, NSH], F16, tag=f"hf{ch}")
                nc.vector.tensor_mul(out=t, in0=h_ps[ch], in1=recip)
                nc.vector.tensor_scalar_add(out=t, in0=t, scalar1=bv[ch])
                hf.append(t)

            # ---- projection + bias + residual ----
            for oh in range(2):
                op = PS.tile([128, NSH], F32, tag="st", bufs=2)
                for nh in range(2):
                    for ch in range(2):
                        nc.tensor.matmul(
                            op[:, nh * 512:(nh + 1) * 512],
                            wpT[ch][:, oh * 128:(oh + 1) * 128],
                            hf[ch][:, nh * 512:(nh + 1) * 512],
                            start=(ch == 0), stop=(ch == 1),
                        )
                osb = W.tile([128, NSH], F32, tag="osb", bufs=2)
                nc.vector.scalar_tensor_tensor(
                    out=osb, in0=op, scalar=bp[oh], in1=xb[oh][:, 0:NSH],
                    op0=Alu.add, op1=Alu.add,
                )
                nc.sync.dma_start(out=d_out[oh], in_=osb)

    nc.compile()
    return nc


def _host_inputs(x, gamma, beta, wq, bq, wk, bk, wv, bv, wp, bp):
    """Build the per-core input maps (list of 8 dicts)."""
    f16 = np.float16
    f32 = np.float32
    xr = np.asarray(x, f32).reshape(2, C, N)

    def wt(w):
        return np.ascontiguousarray(np.asarray(w, f32).T).astype(f16).reshape(2, 128, C)

    def col(v):
        return np.asarray(v, f32).reshape(2, 128, 1)

    gmat = np.kron(np.eye(16, dtype=f32), np.full((8, 8), 1.0 / 8.0, f32))
    ones = np.ones((128, 128), f16)
    common = {
        "wqT": wt(wq), "wkT": wt(wk), "wvT": wt(wv), "wpT": wt(wp),
        "gamma": col(gamma), "beta": col(beta),
        "bq": col(bq), "bk": col(bk), "bv": col(bv), "bp": col(bp),
        "gmat": gmat, "ones": ones,
    }
    in_maps = []
    for core in range(NCORES):
        b, s = divmod(core, 4)
        xb = np.roll(xr[b], -s * NSH, axis=1).reshape(2, 128, N)
        in_maps.append({"xb": np.ascontiguousarray(xb), **common})
    return in_maps


def _gather(results):
    out = np.empty((2, C, N), np.float32)
    for core in range(NCORES):
        b, s = divmod(core, 4)
        out[b, :, s * NSH:(s + 1) * NSH] = results[core]["out"].reshape(C, NSH)
    return out.reshape(2, C, 16, 16, 16)


def kernel(x, gamma, beta, wq, bq, wk, bk, wv, bv, wp, bp):
    from concourse import bass_utils

    if "nc" not in _CACHE:
        _CACHE["nc"] = _build_program()
    nc = _CACHE["nc"]
    in_maps = _host_inputs(x, gamma, beta, wq, bq, wk, bk, wv, bv, wp, bp)
    res = bass_utils.run_bass_kernel_spmd(nc, in_maps, core_ids=list(range(NCORES)))
    return _gather(res.results)


# revision 10
# speedup vs baseline: 1.0826x; 1.0826x over previous
"""Trainium2 Bass kernel for AttnBlock (GroupNorm + QKV + NxN attention + proj + residual).

Contract: kernel(**inputs) takes the FULL unsharded inputs (as produced by
setup_inputs) and returns the FULL output, running on 8 NeuronCores via
bass_utils.run_bass_kernel_spmd.

Sharding: core i handles (batch b = i//4, query-shard s = i%4). The host
rotates x[b] by -s*1024 along the flattened spatial axis so the (identical)
SPMD program always treats columns 0:1024 as its query rows: attention and
GroupNorm are permutation-invariant over key positions, so only the output
column order matters, and out columns 0:1024 of the rotated problem are
exactly out[b][:, s*1024:(s+1)*1024] of the original.

On-chip layout (per core):
  - channels on partitions in 2 halves of 128; spatial (4096) on the free axis
  - GroupNorm stats via bn_stats/bn_aggr per channel, then one block-diagonal
    (1/8) 128x128 fp32 matmul to average the 8 channels of each group, rstd on
    DVE/ACT, applied with ACT passes (per-partition scale+bias)
  - q/k/v weights are passed pre-transposed (c,o) so all matmuls are
    transpose-free; v is produced directly in (m, c) layout
  - scores are computed transposed, S^T[m,n] = sum_c k[c,m] q[c,n], with keys
    m on partitions (32 chunks of 128): softmax over m needs no max pass
    (|scores| <~ 10), exp runs on ACT into fp16, PV accumulates
    h[c,n] += vT[m,c]^T exp[m,n] in PSUM across all 32 chunks
  - softmax denominator: DVE accumulates exp chunks, then a ones(128,128)
    matmul computes column sums broadcast to all partitions in one shot;
    reciprocal_approx_fast; the division commutes with both PV and the final
    projection, so wp runs on unnormalized h and the output is scaled at the
    end — this keeps the reciprocal off the critical path
  - bv is folded into the host-precomputed bias bpp = wp @ bv + bp (softmax
    rows sum to one); bpp and the residual add are fused into a single
    scalar_tensor_tensor
"""

import numpy as np

C = 256
N = 4096  # spatial positions (16*16*16)
NSH = 1024  # query shard per core
NCORES = 8
EPS = 1e-6
SCALE = 1.0 / 16.0  # C ** -0.5

_CACHE = {}


def _build_program():
    import concourse.bass as bass
    import concourse.tile as tile
    from concourse import bacc, mybir

    F32 = mybir.dt.float32
    F16 = mybir.dt.float16
    Alu = mybir.AluOpType
    Act = mybir.ActivationFunctionType

    nc = bacc.Bacc("TRN2", target_bir_lowering=False, debug=False,
                   num_devices=NCORES)

    d_xb = nc.dram_tensor("xb", [2, 128, N], F32, kind="ExternalInput").ap()
    # wall = [wqT | wkT | wvT | wpT] along the free axis, per channel-half
    d_wall = nc.dram_tensor("wall", [2, 128, 4 * C], F16, kind="ExternalInput").ap()
    # cols = [gamma, beta, bq, bk, bpp] per channel-half
    d_cols = nc.dram_tensor("cols", [2, 128, 5], F32, kind="ExternalInput").ap()
    d_gmat = nc.dram_tensor("gmat", [128, 128], F32, kind="ExternalInput").ap()
    d_ones = nc.dram_tensor("ones", [128, 128], F16, kind="ExternalInput").ap()
    d_out = nc.dram_tensor("out", [2, 128, NSH], F32, kind="ExternalOutput").ap()

    MCH = N // 128  # 32 key chunks

    with tile.TileContext(nc) as tc:
        with (
            tc.tile_pool(name="persist", bufs=1) as P,
            tc.tile_pool(name="work", bufs=2) as W,
            tc.tile_pool(name="psum", bufs=1, space="PSUM") as PS,
        ):
            # ---- x loads first: they own the sync (SP HWDGE) ring ----
            xb = []
            for h in range(2):
                t = P.tile([128, N], F32, tag=f"xb{h}")
                xb.append(t)
            for j in range(4):
                for h in range(2):
                    nc.sync.dma_start(
                        out=xb[h][:, j * 1024:(j + 1) * 1024],
                        in_=d_xb[h, :, j * 1024:(j + 1) * 1024],
                    )

            # ---- constants / weights on the gpsimd (SWDGE) ring ----
            gmat = P.tile([128, 128], F32, tag="gmat")
            nc.gpsimd.dma_start(out=gmat, in_=d_gmat)
            ones = P.tile([128, 128], F16, tag="ones")
            nc.gpsimd.dma_start(out=ones, in_=d_ones)
            wall, cols = [], []
            for h in range(2):
                t = P.tile([128, 4 * C], F16, tag=f"wall{h}")
                nc.gpsimd.dma_start(out=t, in_=d_wall[h])
                wall.append(t)
                t = P.tile([128, 5], F32, tag=f"cols{h}")
                nc.gpsimd.dma_start(out=t, in_=d_cols[h])
                cols.append(t)
            wqT = [wall[h][:, 0 * C:1 * C] for h in range(2)]
            wkT = [wall[h][:, 1 * C:2 * C] for h in range(2)]
            wvT = [wall[h][:, 2 * C:3 * C] for h in range(2)]
            wpT = [wall[h][:, 3 * C:4 * C] for h in range(2)]
            gamma = [cols[h][:, 0:1] for h in range(2)]
            beta = [cols[h][:, 1:2] for h in range(2)]
            bq = [cols[h][:, 2:3] for h in range(2)]
            bk = [cols[h][:, 3:4] for h in range(2)]
            bpp = [cols[h][:, 4:5] for h in range(2)]

            eps_t = P.tile([128, 1], F32, tag="eps")
            nc.vector.memset(eps_t, EPS)
            # preload the Sqrt ACT table while the x DMA is in flight
            warm = W.tile([128, 1], F32, tag="warm", bufs=2)
            nc.scalar.activation(out=warm, in_=eps_t, func=Act.Sqrt,
                                 bias=0.0, scale=1.0)

            # ---- GroupNorm stats ----
            hn = []
            for h in range(2):
                stats = W.tile([128, 8, 6], F32, tag="bnstats", bufs=2)
                for j in range(8):
                    nc.vector.bn_stats(
                        out=stats[:, j, :], in_=xb[h][:, j * 512:(j + 1) * 512]
                    )
                mv = W.tile([128, 2], F32, tag="mv", bufs=2)
                nc.vector.bn_aggr(out=mv, in_=stats)

                # cm = [mean, mean^2 + var] per channel
                cm = W.tile([128, 2], F32, tag="cm", bufs=2)
                nc.vector.tensor_copy(out=cm[:, 0:1], in_=mv[:, 0:1])
                nc.vector.scalar_tensor_tensor(
                    out=cm[:, 1:2], in0=mv[:, 0:1], scalar=mv[:, 0:1],
                    in1=mv[:, 1:2], op0=Alu.mult, op1=Alu.add,
                )
                # per-channel group stats: [mean_g, E_g[x^2]] (gmat has the /8)
                gst = PS.tile([128, 2], F32, tag="st", bufs=2)
                nc.tensor.matmul(gst, gmat, cm)

                gsb = W.tile([128, 2], F32, tag="gsb", bufs=2)
                nc.vector.tensor_copy(out=gsb, in_=gst)
                msq = W.tile([128, 1], F32, tag="msq", bufs=2)
                nc.vector.tensor_mul(out=msq, in0=gsb[:, 0:1], in1=gsb[:, 0:1])
                varg = W.tile([128, 1], F32, tag="varg", bufs=2)
                nc.vector.tensor_sub(out=varg, in0=gsb[:, 1:2], in1=msq)
                sd = W.tile([128, 1], F32, tag="sd", bufs=2)
                nc.scalar.activation(out=sd, in_=varg, func=Act.Sqrt,
                                     bias=eps_t, scale=1.0)
                rstd = W.tile([128, 1], F32, tag="rstd", bufs=2)
                nc.vector.reciprocal(out=rstd, in_=sd)
                s_t = P.tile([128, 1], F32, tag=f"s{h}")
                nc.vector.tensor_mul(out=s_t, in0=rstd, in1=gamma[h])
                ms = W.tile([128, 1], F32, tag="ms", bufs=2)
                nc.vector.tensor_mul(out=ms, in0=gsb[:, 0:1], in1=s_t)
                t_t = P.tile([128, 1], F32, tag=f"t{h}")
                nc.vector.tensor_sub(out=t_t, in0=beta[h], in1=ms)

                # apply: hn = x * s + t  (fp16), 4 chunks for QKV overlap
                ht = P.tile([128, N], F16, tag=f"hn{h}")
                for j in range(4):
                    nc.scalar.activation(
                        out=ht[:, j * 1024:(j + 1) * 1024],
                        in_=xb[h][:, j * 1024:(j + 1) * 1024],
                        func=Act.Identity, bias=t_t, scale=s_t,
                    )
                hn.append(ht)

            # ---- q (only shard columns 0:NSH) ----
            q_sb = []
            for oh in range(2):
                qp = PS.tile([128, NSH], F32, tag="st", bufs=2)
                for nh in range(2):
                    for ch in range(2):
                        nc.tensor.matmul(
                            qp[:, nh * 512:(nh + 1) * 512],
                            wqT[ch][:, oh * 128:(oh + 1) * 128],
                            hn[ch][:, nh * 512:(nh + 1) * 512],
                            start=(ch == 0), stop=(ch == 1),
                        )
                qs = P.tile([128, NSH], F16, tag=f"q{oh}")
                if oh == 0:
                    nc.scalar.activation(out=qs, in_=qp, func=Act.Identity,
                                         bias=bq[oh])
                else:
                    nc.vector.tensor_scalar_add(out=qs, in0=qp, scalar1=bq[oh])
                q_sb.append(qs)

            # ---- k (full 4096); copies alternate ACT/DVE ----
            k_sb = []
            for oh in range(2):
                ks = P.tile([128, N], F16, tag=f"k{oh}")
                for mt in range(4):
                    kp = PS.tile([128, 1024], F32, tag="st", bufs=2)
                    for nh in range(2):
                        for ch in range(2):
                            nc.tensor.matmul(
                                kp[:, nh * 512:(nh + 1) * 512],
                                wkT[ch][:, oh * 128:(oh + 1) * 128],
                                hn[ch][:, mt * 1024 + nh * 512:
                                        mt * 1024 + (nh + 1) * 512],
                                start=(ch == 0), stop=(ch == 1),
                            )
                    dst = ks[:, mt * 1024:(mt + 1) * 1024]
                    if mt % 2 == 0:
                        nc.scalar.activation(out=dst, in_=kp, func=Act.Identity,
                                             bias=bk[oh])
                    else:
                        nc.vector.tensor_scalar_add(out=dst, in0=kp,
                                                    scalar1=bk[oh])
                k_sb.append(ks)

            # ---- vT: (m, c) layout, computed directly (bias folded in bpp)
            vt = P.tile([128, MCH * C], F16, tag="vt")
            for mc in range(MCH):
                vp = PS.tile([128, C], F32, tag="st", bufs=2)
                for ch in range(2):
                    nc.tensor.matmul(
                        vp, hn[ch][:, mc * 128:(mc + 1) * 128], wvT[ch],
                        start=(ch == 0), stop=(ch == 1),
                    )
                dst = vt[:, mc * C:(mc + 1) * C]
                if mc % 3 == 2:
                    nc.scalar.copy(out=dst, in_=vp)
                else:
                    nc.vector.tensor_copy(out=dst, in_=vp)

            # ---- attention: S^T chunks, exp, PV, denominator acc ----
            dacc = P.tile([128, NSH], F16, tag="dacc")
            h_ps = [PS.tile([128, NSH], F32, tag=f"h{ch}", bufs=1,
                            name=f"h_ps{ch}")
                    for ch in range(2)]
            for mc in range(MCH):
                st = PS.tile([128, NSH], F32, tag="st", bufs=2)
                for nh in range(2):
                    for ch in range(2):
                        nc.tensor.matmul(
                            st[:, nh * 512:(nh + 1) * 512],
                            k_sb[ch][:, mc * 128:(mc + 1) * 128],
                            q_sb[ch][:, nh * 512:(nh + 1) * 512],
                            start=(ch == 0), stop=(ch == 1),
                        )
                ex = W.tile([128, NSH], F16, tag="ex", bufs=4)
                nc.scalar.activation(out=ex, in_=st, func=Act.Exp, scale=SCALE)
                for ch in range(2):
                    for nh in range(2):
                        nc.tensor.matmul(
                            h_ps[ch][:, nh * 512:(nh + 1) * 512],
                            vt[:, mc * C + ch * 128: mc * C + (ch + 1) * 128],
                            ex[:, nh * 512:(nh + 1) * 512],
                            start=(mc == 0), stop=(mc == MCH - 1),
                        )
                if mc == 0:
                    nc.vector.tensor_copy(out=dacc, in_=ex)
                else:
                    nc.vector.tensor_add(out=dacc, in0=dacc, in1=ex)

            # ---- unnormalized h -> fp16 for the projection ----
            hr = []
            for ch in range(2):
                t = P.tile([128, NSH], F16, tag=f"hr{ch}")
                nc.vector.tensor_copy(out=t, in_=h_ps[ch])
                hr.append(t)

            # ---- denominator: colsum broadcast, fast reciprocal ----
            den = PS.tile([128, NSH], F32, tag="st", bufs=2)
            for nh in range(2):
                nc.tensor.matmul(den[:, nh * 512:(nh + 1) * 512], ones,
                                 dacc[:, nh * 512:(nh + 1) * 512])
            recip = P.tile([128, NSH], F32, tag="recip")
            nc.vector.reciprocal_approx_fast(out=recip, in_=den)

            # ---- projection on unnormalized h, then scale + bias + residual
            for oh in range(2):
                op = PS.tile([128, NSH], F32, tag="st", bufs=2)
                for nh in range(2):
                    for ch in range(2):
                        nc.tensor.matmul(
                            op[:, nh * 512:(nh + 1) * 512],
                            wpT[ch][:, oh * 128:(oh + 1) * 128],
                            hr[ch][:, nh * 512:(nh + 1) * 512],
                            start=(ch == 0), stop=(ch == 1),
                        )
                tmp = W.tile([128, NSH], F32, tag="tmp", bufs=2)
                nc.vector.tensor_mul(out=tmp, in0=op, in1=recip)
                osb = W.tile([128, NSH], F32, tag="osb", bufs=2)
                nc.vector.scalar_tensor_tensor(
                    out=osb, in0=tmp, scalar=bpp[oh], in1=xb[oh][:, 0:NSH],
                    op0=Alu.add, op1=Alu.add,
                )
                nc.sync.dma_start(out=d_out[oh], in_=osb)

    nc.compile()
    return nc


def _host_inputs(x, gamma, beta, wq, bq, wk, bk, wv, bv, wp, bp):
    """Build the per-core input maps (list of 8 dicts)."""
    f16 = np.float16
    f32 = np.float32
    xr = np.asarray(x, f32).reshape(2, C, N)

    def wt(w):
        return np.ascontiguousarray(np.asarray(w, f32).T).astype(f16)

    wall = np.concatenate([wt(wq), wt(wk), wt(wv), wt(wp)], axis=1)
    wall = np.ascontiguousarray(wall.reshape(2, 128, 4 * C))

    bpp = np.asarray(wp, f32) @ np.asarray(bv, f32) + np.asarray(bp, f32)
    cols = np.stack(
        [np.asarray(v, f32) for v in (gamma, beta, bq, bk, bpp)], axis=1
    ).reshape(2, 128, 5)
    cols = np.ascontiguousarray(cols)

    gmat = np.kron(np.eye(16, dtype=f32), np.full((8, 8), 1.0 / 8.0, f32))
    ones = np.ones((128, 128), f16)
    common = {"wall": wall, "cols": cols, "gmat": gmat, "ones": ones}
    in_maps = []
    for core in range(NCORES):
        b, s = divmod(core, 4)
        xb = np.roll(xr[b], -s * NSH, axis=1).reshape(2, 128, N)
        in_maps.append({"xb": np.ascontiguousarray(xb), **common})
    return in_maps


def _gather(results):
    out = np.empty((2, C, N), np.float32)
    for core in range(NCORES):
        b, s = divmod(core, 4)
        out[b, :, s * NSH:(s + 1) * NSH] = results[core]["out"].reshape(C, NSH)
    return out.reshape(2, C, 16, 16, 16)


def kernel(x, gamma, beta, wq, bq, wk, bk, wv, bv, wp, bp):
    from concourse import bass_utils

    if "nc" not in _CACHE:
        _CACHE["nc"] = _build_program()
    nc = _CACHE["nc"]
    in_maps = _host_inputs(x, gamma, beta, wq, bq, wk, bk, wv, bv, wp, bp)
    res = bass_utils.run_bass_kernel_spmd(nc, in_maps, core_ids=list(range(NCORES)))
    return _gather(res.results)
